# revision 2
# baseline (speedup 1.0000x reference)
"""KMeans summarize kernel for Trainium2 — v2: engine-balanced argmin.

Per 128-point tile: bf16 dist matmul -> PSUM q = CC - 2Xc (shifted).
Tiles are processed in pairs, each pair one of two types:
  B: DVE tensor_scalar drains q to fp32 SBUF with fused accum-min, then
     DVE is_le (exact, fp32) emits a {1,0} fp8 one-hot.
  C: ACT Copy drains q to fp16 SBUF, DVE tensor_scalar fast-min (4x mode)
     accumulates the min, DVE (q16 - m) <= delta emits the fp8 one-hot.
Scatter: fp8 DoubleRow matmul per pair (mh packed [P,2,128] fp8) into a
single PSUM accumulator; no sign correction needed for {1,0} one-hots.
Inertia from per-tile mins + XX shipped separately.
"""

import sys

sys.path.insert(0, "/opt/trn_rl_repo")

import numpy as np

N, D, K = 500_000, 64, 512
NCORES = 8
P = 128
TILES = 490                      # per-core tiles (even)
PAIRS = TILES // 2               # 245
ROWS = P * TILES                 # 62720 per core
NPAD = NCORES * ROWS             # 501760
GROUP = 8                        # tiles per xt DMA slab
MGROUP = 4                       # pairs per mh DMA slab
CROWS = D + 2                    # dist contraction rows: X(64) + 1 + 1

SHIFT = 28.0                     # q = CC - SHIFT - 2Xc  (recentres min near 0)
DELTA = 0.03                     # C-tile compare margin (fp16 rounding cover)
B_EVERY = 7                      # pair pr is type B iff pr % B_EVERY == 0
PEND_PAIRS = 4                   # scatter matmuls lag this many pairs
DPP_BUFS = 3                     # dist PSUM pair-tiles in flight (2 banks each)

_CACHE = {}


def _pair_type(pr: int) -> str:
    return "B" if pr % B_EVERY == 0 else "C"


def _build():
    import concourse.bass as bass
    import concourse.mybir as mybir
    import concourse.tile as tile

    fp32 = mybir.dt.float32
    bf16 = mybir.dt.bfloat16
    fp16 = mybir.dt.float16
    fp8 = mybir.dt.float8e4

    nc = bass.Bass()
    xt = nc.dram_tensor("xt", (CROWS, ROWS), bf16, kind="ExternalInput")
    rhs = nc.dram_tensor("rhs", (CROWS, K), bf16, kind="ExternalInput")
    mh8 = nc.dram_tensor("mh8", (P, PAIRS, 2, 2 * D), fp8, kind="ExternalInput")
    xxb = nc.dram_tensor("xxb", (P, TILES), fp32, kind="ExternalInput")
    out = nc.dram_tensor("out", (P, K + 4), fp32, kind="ExternalOutput")

    AX = mybir.AxisListType.X
    OP = mybir.AluOpType
    AF = mybir.ActivationFunctionType
    DR = mybir.MatmulPerfMode.DoubleRow

    from contextlib import ExitStack

    with tile.TileContext(nc) as tc, ExitStack() as es:
        consts = es.enter_context(tc.tile_pool(name="consts", bufs=1))
        xtpA = es.enter_context(tc.tile_pool(name="xtpA", bufs=2))
        xtpB = es.enter_context(tc.tile_pool(name="xtpB", bufs=2))
        mhpA = es.enter_context(tc.tile_pool(name="mhpA", bufs=2))
        mhpB = es.enter_context(tc.tile_pool(name="mhpB", bufs=2))
        qBp = es.enter_context(tc.tile_pool(name="qBp", bufs=3))
        qCp = es.enter_context(tc.tile_pool(name="qCp", bufs=4))
        a8p = es.enter_context(tc.tile_pool(name="a8p", bufs=6))
        scrp = es.enter_context(tc.tile_pool(name="scrp", bufs=2))
        dpp = es.enter_context(tc.tile_pool(name="dpp", bufs=DPP_BUFS, space="PSUM"))
        scp = es.enter_context(tc.tile_pool(name="scp", bufs=1, space="PSUM"))

        rhs_sb = consts.tile([CROWS, K], bf16)
        nc.sync.dma_start(out=rhs_sb, in_=rhs.ap())
        xxb_sb = consts.tile([P, TILES], fp32)
        nc.sync.dma_start(out=xxb_sb, in_=xxb.ap())
        mbuf = consts.tile([P, TILES], fp32)

        scat = scp.tile([P, K], fp32)          # single {1,0} accumulator

        n_xslab = (TILES + GROUP - 1) // GROUP
        n_mslab = (PAIRS + MGROUP - 1) // MGROUP
        xt_slabs = [None] * n_xslab
        mh_slabs = [None] * n_mslab

        pend = []        # (mh_sb, pj, a8pair, pr)

        def flush_pend(k):
            for _ in range(k):
                pmh, ppj, pa8, ppr = pend.pop(0)
                nc.tensor.matmul(
                    scat, pmh[:, ppj, :, :], pa8,
                    start=(ppr == 0), stop=(ppr == PAIRS - 1),
                    perf_mode=DR,
                )

        for pr in range(PAIRS):
            t0 = 2 * pr
            # ---- slab loads ----
            gx = t0 // GROUP
            if xt_slabs[gx] is None:
                g = min(GROUP, TILES - gx * GROUP)
                xtg = (xtpA if gx % 2 == 0 else xtpB).tile(
                    [CROWS, GROUP * P], bf16, tag="xtg", name="xtg"
                )
                src = bass.AP(
                    tensor=xt, offset=gx * GROUP * P,
                    ap=[[ROWS, CROWS], [1, g * P]],
                )
                nc.sync.dma_start(out=xtg[:, : g * P], in_=src)
                xt_slabs[gx] = xtg
            gm = pr // MGROUP
            if mh_slabs[gm] is None:
                g = min(MGROUP, PAIRS - gm * MGROUP)
                mhg = (mhpA if gm % 2 == 0 else mhpB).tile(
                    [P, MGROUP, 2, 2 * D], fp8, tag="mhg", name="mhg"
                )
                src = bass.AP(
                    tensor=mh8, offset=gm * MGROUP * 2 * 2 * D,
                    ap=[[PAIRS * 2 * 2 * D, P], [1, g * 2 * 2 * D]],
                )
                nc.sync.dma_start(out=mhg[:, :g, :, :], in_=src)
                mh_slabs[gm] = mhg

            xtg = xt_slabs[gx]
            mhg = mh_slabs[gm]

            # ---- dist matmuls for the pair ----
            dd = dpp.tile([P, 2, K], fp32, tag="dist")
            for h in (0, 1):
                t = t0 + h
                j = t - gx * GROUP
                nc.tensor.matmul(
                    dd[:, h, :], xtg[:, j * P:(j + 1) * P], rhs_sb,
                    start=True, stop=True,
                )

            a8 = a8p.tile([P, 2, K], fp8, tag="a8", name="a8")
            if _pair_type(pr) == "B":
                qB = qBp.tile([P, 2, K], fp32, tag="qB", name="qB")
                for h in (0, 1):
                    t = t0 + h
                    nc.vector.tensor_scalar(
                        out=qB[:, h, :], in0=dd[:, h, :],
                        scalar1=0.0, scalar2=None,
                        op0=OP.add, op1=OP.min,
                        accum_out=mbuf[:, t:t + 1],
                    )
                for h in (0, 1):
                    t = t0 + h
                    nc.vector.tensor_scalar(
                        out=a8[:, h, :], in0=qB[:, h, :],
                        scalar1=mbuf[:, t:t + 1], scalar2=None,
                        op0=OP.is_le,
                    )
            else:
                qC = qCp.tile([P, 2, K], fp16, tag="qC", name="qC")
                for h in (0, 1):
                    nc.scalar.activation(
                        out=qC[:, h, :], in_=dd[:, h, :], func=AF.Copy,
                    )
                scr = scrp.tile([P, K], fp16, tag="scr", name="scr")
                for h in (0, 1):
                    t = t0 + h
                    nc.vector.tensor_scalar(
                        out=scr, in0=qC[:, h, :],
                        scalar1=0.0, scalar2=None,
                        op0=OP.add, op1=OP.min,
                        accum_out=mbuf[:, t:t + 1],
                    )
                for h in (0, 1):
                    t = t0 + h
                    nc.vector.tensor_scalar(
                        out=a8[:, h, :], in0=qC[:, h, :],
                        scalar1=mbuf[:, t:t + 1], scalar2=DELTA,
                        op0=OP.subtract, op1=OP.is_le,
                    )

            pend.append((mhg, pr - gm * MGROUP, a8, pr))
            if len(pend) > PEND_PAIRS:
                flush_pend(len(pend) - PEND_PAIRS)
        flush_pend(len(pend))

        # ---- finalize ----
        out_sb = consts.tile([P, K + 4], fp32)
        nc.vector.tensor_copy(out_sb[:, :K], scat)
        vbuf = consts.tile([P, TILES], fp32)
        nc.vector.tensor_tensor(
            out=vbuf, in0=mbuf, in1=xxb_sb, op=OP.add
        )
        nc.vector.tensor_scalar_max(vbuf, vbuf, 0.0)
        vsq = consts.tile([P, TILES], fp32)
        nc.scalar.activation(
            out=vsq, in_=vbuf, func=AF.Sqrt, scale=1.0 / D,
            accum_out=out_sb[:, K:K + 1],
        )
        nc.vector.memset(out_sb[:, K + 1:], 0.0)
        nc.sync.dma_start(out=out.ap(), in_=out_sb)

    _split_multi_waits(nc, mybir)
    return nc


def _split_multi_waits(nc, mybir):
    """Walrus allows max 1 sem-wait per instruction: hoist extras onto
    inserted NoOps on the same engine queue."""
    import copy

    module = nc.m
    new_module = copy.replace(module, functions=[])
    for function in module.functions:
        new_function = copy.replace(function, blocks=[])
        new_function.set_allocations_from_list(function.allocations)
        for block in function.blocks:
            new_insts = []
            for ins in block.instructions:
                si = ins.sync_info
                if si is not None and si.on_wait and len(si.on_wait) > 1:
                    waits = list(si.on_wait)
                    for k, w in enumerate(waits[:-1]):
                        new_insts.append(mybir.InstNoOp(
                            name=f"{ins.name}-wsplit{k}", engine=ins.engine,
                            ins=[], outs=[],
                            sync_info=mybir.SyncInfo(on_wait=[w], on_update=[]),
                        ))
                    ins.sync_info = mybir.SyncInfo(
                        on_wait=[waits[-1]], on_update=list(si.on_update or [])
                    )
                new_insts.append(ins)
            new_function.blocks.append(copy.replace(block, instructions=new_insts))
        new_module.functions.append(new_function)
    nc.m = new_module


def _prep_inputs(X, centroids, sample_weight):
    import ml_dtypes

    bf16 = ml_dtypes.bfloat16
    f8 = ml_dtypes.float8_e4m3

    C = np.asarray(centroids, dtype=np.float32)
    X = np.asarray(X, dtype=np.float32)
    W = np.asarray(sample_weight, dtype=np.float32)

    CC = (C * C).sum(axis=1) - SHIFT                     # (K,)
    CChi = CC.astype(bf16)
    CClo = (CC - CChi.astype(np.float32)).astype(bf16)
    rhs = np.empty((CROWS, K), dtype=bf16)
    rhs[:D] = (-2.0 * C.T).astype(bf16)
    rhs[D] = CChi
    rhs[D + 1] = CClo

    Xp = np.empty((NPAD, D), dtype=np.float32)
    Xp[:N] = X
    Xp[N:] = C[0]
    Wp = np.zeros((NPAD, D), dtype=np.float32)
    Wp[:N] = W
    XXp = np.einsum("ij,ij->i", Xp, Xp) + SHIFT

    Mh = np.empty((NPAD, 2 * D), dtype=f8)
    Mh[:, :D] = (Xp * Wp).astype(f8)
    Mh[:, D:] = Wp.astype(f8)

    Xb = Xp.astype(bf16)
    in_maps = []
    for c in range(NCORES):
        sl = slice(c * ROWS, (c + 1) * ROWS)
        xtc = np.empty((CROWS, ROWS), dtype=bf16)
        xtc[:D] = Xb[sl].T
        xtc[D] = bf16(1.0)
        xtc[D + 1] = bf16(1.0)
        # mh8: (P, PAIRS, 2, 2D) with point (pr*2+i)*P + p -> [p, pr, i, :]
        mh_c = np.ascontiguousarray(
            Mh[sl].reshape(PAIRS, 2, P, 2 * D).transpose(2, 0, 1, 3)
        )
        xx_c = np.ascontiguousarray(
            XXp[sl].reshape(TILES, P).T.astype(np.float32)
        )
        in_maps.append({"xt": xtc, "rhs": rhs, "mh8": mh_c, "xxb": xx_c})
    return in_maps


def run(X, centroids, sample_weight, trace=False):
    from concourse.bass_utils import run_bass_kernel_spmd

    if "nc" not in _CACHE:
        _CACHE["nc"] = _build()
    in_maps = _prep_inputs(X, centroids, sample_weight)
    res = run_bass_kernel_spmd(
        _CACHE["nc"], in_maps, core_ids=list(range(NCORES)), trace=trace
    )
    xw = np.zeros((K, D), dtype=np.float64)
    ws = np.zeros((K, D), dtype=np.float64)
    inertia = 0.0
    for c in range(NCORES):
        o = res.results[c]["out"]
        xw += o[:D, :K].T.astype(np.float64)
        ws += o[D:2 * D, :K].T.astype(np.float64)
        inertia += float(o[:, K].sum(dtype=np.float64))
    packed = np.concatenate(
        [xw, ws, np.full((1, D), inertia)], axis=0
    ).astype(np.float32)
    return packed, res


def kernel(X, centroids, sample_weight):
    packed, _ = run(X, centroids, sample_weight)
    return packed
